# revision 42
# baseline (speedup 1.0000x reference)
"""Trainium2 Bass kernel for nn_Attention_45037027066352 (sparse_attention).

Reference computation (per batch b, head h; N=1024 tokens, HD=64, H=12):
    qkv   = x @ Wqkv.T                     -> q,k,v [B,H,N,HD]
    Qspk  = relu(q) @ Wfc1.T + bfc1
    Kspk  = relu(k) @ Wfc2.T + bfc2
    att   = softmax(relu(Qspk @ Kspk.T * SCALE) * 2)
    out_h = att @ (relu(v) * 4)
    y     = concat_h(out_h) @ Wproj.T + bproj
Sharding: pure data-parallel over B=8 across the 8 NeuronCores.

Key numeric identity exploited here: with z the scaled scores,
    P := exp(relu(z)) = 1 + ptil,   ptil := relu(exp(z) - 1)
    att @ Vr = (ptil @ Vr + colsum(Vr)) / (rowsum(ptil) + N)
ptil in [0, ~0.09] keeps full fp8e4m3 relative precision (storing P itself
would quantize the entire attention signal into one fp8 step near 1.0).

Schedule (per core): six windows, window w streams pair w's 16 exps
(~21us ACT each) while the PE runs the window's slab list (scores JIT,
qk/fc of pair w+1, PV of pair w-1, v/colsum/proj). ACT exp of the 12.6M
scores is the pacing engine; everything else hides under it.

PE work reductions vs the A/B-packed f32 design:
  - ptil and Vr live in fp8; the PV product runs DoubleRow matmuls that
    contract TWO 128-token tiles per instruction (256 rows), so a head's
    PV chain is 4 matmuls instead of 8, at the same 512-cycle stream.
  - the V operand carries a 65th column of ones, so row 64 of each PV
    PSUM tile is the rowsum(ptil) for free: the entire separate
    ones-matmul rowsum pass (~20us PE) is deleted.
  - colsum(Vr) is 2 ones-lhsT chains over the vo buffer (one [1,960] row),
    bounced through DRAM into a [65,12] per-head column that the PV
    copyback adds per-partition (rows 0:64 = colsum, row 64 = 0; the +N
    for the rowsum is added to the reciprocal input instead).

Dataflow per (pair, i-half, head): 4 DR matmuls -> [65,512] PSUM ->
tensor_scalar(+cva) -> bf16 staging -> DMA rows 0:64 to outT (head B's
rows land at partitions 64:128 via the DMA hop; engines cannot cross
partitions) and row 64 to rs_dram. Reciprocal: rs_dram [2,1024] read back
as [128,16], +1024, recip, broadcast to recb [128,1024] (A rows 0:64, B
64:128); outT normalized in place by a bf16 2x tensor_tensor before proj.

TRN2 Matmult instructions encode at most ONE sync wait, so every matmul's
dependencies either are pre-observed by the PE (gate matmuls per DMA
queue) or land on a single engine semaphore:
  - weight/x DMAs are "gated" by a tiny PE matmul reading them,
  - accumulator-pool consumers are all DVE, score-tile slot handoffs are
    all ACT, so each matmul carries exactly one wait.
DMA queues: sync (SP) carries consts/x/bounces/staging/output; the idle
GPSIMD engine's SWDGE queue carries all weight loads in parallel.
"""

import numpy as np

import concourse.bass as bass
import concourse.bacc as bacc_mod
import concourse.mybir as mybir
import concourse.tile as tile
from concourse.bass_utils import run_bass_kernel_spmd

import ml_dtypes

B, N, C, H, HD = 8, 1024, 768, 12, 64
SCALE = HD**-0.5
T_STEPS = 4

F32 = mybir.dt.float32
BF16 = mybir.dt.bfloat16
FP8 = mybir.dt.float8e4

NPAIR = H // 2  # 6 head pairs
KC = C // 128  # 6 contraction chunks for C=768
NT = N // 128  # 8 token tiles
NH = N // 512  # 2 free-dim halves
DR = mybir.MatmulPerfMode.DoubleRow


def build_nc() -> bass.Bass:
    nc = bacc_mod.Bacc()

    xT = nc.dram_tensor("xT", [C, N], FP8, kind="ExternalInput")
    wqkvT = nc.dram_tensor("wqkvT", [C, 3 * C], FP8, kind="ExternalInput")
    wfc1p = nc.dram_tensor("wfc1p", [128, 128], BF16, kind="ExternalInput")
    wfc2p = nc.dram_tensor("wfc2p", [128, 128], BF16, kind="ExternalInput")
    b1p = nc.dram_tensor("b1p", [128, 1], F32, kind="ExternalInput")
    b2p = nc.dram_tensor("b2p", [128, 1], F32, kind="ExternalInput")
    wprojT = nc.dram_tensor("wprojT", [C, C], BF16, kind="ExternalInput")
    bprojp = nc.dram_tensor("bprojp", [128, KC], F32, kind="ExternalInput")

    yT = nc.dram_tensor("yT", [C, N], BF16, kind="ExternalOutput")

    # scratch: rowsum/reciprocal reshape bounce + colsum transpose bounce
    rs_dram = nc.dram_tensor("rs_scratch", [NPAIR, 2, N], BF16)
    rec_dram = nc.dram_tensor("rec_scratch", [NPAIR, 2, N], BF16)
    cv_dram = nc.dram_tensor("cv_scratch", [H, 80], F32)

    xT_v = xT.rearrange("(ko p) n -> p ko n", p=128)
    wqkvT_m = wqkvT.rearrange("(ko p) (m j) -> p ko m j", p=128, j=128)
    wqkvT_v = wqkvT.rearrange("(ko p) j -> p ko j", p=128)
    wprojT_v = wprojT.rearrange("(ko p) e -> p ko e", p=128)
    yT_v = yT.rearrange("(eo p) n -> p eo n", p=128)

    with tile.TileContext(nc) as tc:
        with (
            tc.tile_pool(name="consts", bufs=1) as consts,
            tc.tile_pool(name="psum", bufs=3, space="PSUM") as psum,
            tc.tile_pool(name="acc", bufs=2, space="PSUM") as acc_pool,
            tc.tile_pool(name="xin", bufs=1) as x_pool,
            tc.tile_pool(name="wqk", bufs=1) as wqk_pool,
            tc.tile_pool(name="wv", bufs=1) as wv_pool,
            tc.tile_pool(name="wproj", bufs=1) as wproj_pool,
            tc.tile_pool(name="vo", bufs=1) as vo_pool,
            tc.tile_pool(name="rqk", bufs=1) as rqk_pool,
            tc.tile_pool(name="spk", bufs=4) as spk_pool,
            tc.tile_pool(name="texp", bufs=3) as t_pool,
            tc.tile_pool(name="pt", bufs=4) as pt_pool,
            tc.tile_pool(name="outT", bufs=1) as outT_pool,
            tc.tile_pool(name="stg", bufs=4) as stg_pool,
            tc.tile_pool(name="rsmisc", bufs=2) as rs_pool,
            tc.tile_pool(name="yt", bufs=2) as y_pool,
        ):
            trash_holder = [acc_pool.tile([128, 512], F32, tag="acc", name="trash")]

            def acc_tile():
                # accumulator PSUM tile: every consumer's inputs AND the
                # slot handoff land on the DVE monotonic semaphore, so the
                # first matmul needs only one wait.
                return acc_pool.tile([128, 512], F32, tag="acc", name="acct")

            def ps_tile(memset=True):
                t = psum.tile([128, N], F32, tag="ps")
                if memset:
                    nc.vector.memset(t[:, 0:1], 0.0)
                return t

            def gate(region, kpart=128):
                # Tiny PE matmul reading a freshly DMA'd SBUF region so the
                # PE observes that DMA queue's semaphore once.
                m = 65 if kpart == 128 else 64
                nc.tensor.matmul(
                    trash_holder[0][0:m, 0:2],
                    lhsT=region[0:kpart, 0:m],
                    rhs=region[0:kpart, 0:2],
                    start=True,
                    stop=True,
                )

            # ---- SBUF buffers ----
            wfc1_sb = consts.tile([128, 128], BF16)  # blockdiag(Wfc1.T*2s, ..)
            wfc2_sb = consts.tile([128, 128], BF16)
            b1_sb = consts.tile([128, 1], F32)
            b2_sb = consts.tile([128, 1], F32)
            bproj_sb = consts.tile([128, KC], F32)
            ones8_sb = consts.tile([128, 1], FP8)
            cva_sb = consts.tile([128, H], F32)  # rows 0:64 colsum, row 64 = 0

            # x split into kc-pair tiles, one per DMA queue (the tile
            # framework serializes cross-queue writers of a shared tile)
            xA_sb = x_pool.tile([128, 2, N], FP8)
            xB_sb = x_pool.tile([128, 2, N], FP8)
            xC_sb = x_pool.tile([128, 2, N], FP8)
            x_parts = (xA_sb, xB_sb, xC_sb)
            wq0_sb = wqk_pool.tile([128, KC, 128], FP8)
            wk0_sb = wqk_pool.tile([128, KC, 128], FP8)
            wqkR_sb = wqk_pool.tile([128, 2 * NPAIR - 2, KC, 128], FP8)
            wv_sb = wv_pool.tile([128, KC, C], FP8)
            wp_sb = wproj_pool.tile([128, KC, C], BF16)
            # relu(v)*4 in fp8, head-major blocks of 80 (64 vals + ones col
            # at 64 + pad), natural [token, .] layout
            vo_sb = vo_pool.tile([128, NT, H, 80], FP8)
            rqk_sb = rqk_pool.tile([128, 2 * NPAIR, N], BF16)
            outT_sb = outT_pool.tile([128, NPAIR, N], BF16)
            y1_sb = outT_pool.tile([128, KC, N], BF16)  # proj partial kc 0..4

            # ---- DMA issue order ----
            # sync (SP) queue (in-order, ~0.65us issue cadence per DMA):
            # exactly the prelude's critical path — consts, pair-0 qk
            # weights, then x in halves. One gate per queue POINT covers
            # everything before it on that queue.
            # gpsimd SWDGE queue: the bulk weights as 4 batched DMAs.
            # a single HWDGE queue moves only ~65GB/s, so the prelude's
            # critical data (x + pair-0 weights) spreads over all three
            # queues, h0 columns first.
            nc.sync.dma_start(wfc1_sb[:], wfc1p[:, :])
            nc.sync.dma_start(b1_sb[:], b1p[:, :])
            nc.sync.dma_start(b2_sb[:], b2p[:, :])
            nc.sync.dma_start(xA_sb[:, :, 0:512], xT_v[:, 0:2, 0:512])
            nc.sync.dma_start(xA_sb[:, :, 512:N], xT_v[:, 0:2, 512:N])
            nc.sync.dma_start(wfc2_sb[:], wfc2p[:, :])
            nc.sync.dma_start(bproj_sb[:], bprojp[:, :])
            nc.scalar.dma_start(wq0_sb[:], wqkvT_m[:, :, 0, :])
            nc.scalar.dma_start(wk0_sb[:], wqkvT_m[:, :, NPAIR, :])
            nc.scalar.dma_start(xB_sb[:, :, 0:512], xT_v[:, 2:4, 0:512])
            nc.scalar.dma_start(xB_sb[:, :, 512:N], xT_v[:, 2:4, 512:N])
            nc.gpsimd.dma_start(xC_sb[:, :, 0:512], xT_v[:, 4:KC, 0:512])
            nc.gpsimd.dma_start(xC_sb[:, :, 512:N], xT_v[:, 4:KC, 512:N])
            nc.gpsimd.dma_start(wv_sb[:], wqkvT_v[:, :, 2 * C : 3 * C])
            nc.gpsimd.dma_start(
                wqkR_sb[:, 0 : NPAIR - 1], wqkvT_m[:, :, 1:NPAIR, :]
            )
            nc.gpsimd.dma_start(
                wqkR_sb[:, NPAIR - 1 : 2 * NPAIR - 2],
                wqkvT_m[:, :, NPAIR + 1 : 2 * NPAIR, :],
            )
            nc.gpsimd.dma_start(wp_sb[:], wprojT_v[:, :, :])


            nc.vector.memset(ones8_sb[:], 1.0)
            # DR rowsum lhsT for the tail: cols 0:64 ones (head A -> rows
            # 0:64), 64:128 zero, 128:192 ones (head B view [64:192] puts
            # ones at its cols 64:128 -> rows 64:128)
            n1024_sb = consts.tile([128, 1], F32)
            nc.vector.memset(n1024_sb[:], float(N))
            ones2_sb = consts.tile([128, 2, 192], FP8)
            nc.vector.memset(ones2_sb[:], 0.0)
            with nc.allow_low_precision(reason="ones are exact in fp8"):
                nc.vector.memset(ones2_sb[:, :, 0:64], 1.0)
                nc.vector.memset(ones2_sb[:, :, 128:192], 1.0)
                # only the ones column + pad (cols 64:80) need init
                nc.vector.memset(vo_sb[:, :, :, HD:80], 1.0)

            # load the exp table set early (one-time ~2.7us)
            warm_sb = consts.tile([128, 2], F32)
            nc.scalar.activation(
                warm_sb[:], b1_sb[:, 0:1].to_broadcast([128, 2]),
                mybir.ActivationFunctionType.Exp,
            )

            # ---- emission helpers ----
            def wqk_pair(m, kk):
                # [128, 2, 128] weight view for kc pair kk of head block m
                if m == 0:
                    return wq0_sb[:, 2 * kk : 2 * kk + 2]
                if m == NPAIR:
                    return wk0_sb[:, 2 * kk : 2 * kk + 2]
                idx = (m - 1) if m < NPAIR else (m - 2)
                return wqkR_sb[:, idx, 2 * kk : 2 * kk + 2]

            def emit_qk_half(m, h):
                sl = slice(h * 512, (h + 1) * 512)
                qk_ps = acc_tile()
                for kk in range(3):
                    nc.tensor.matmul(
                        qk_ps[:],
                        lhsT=wqk_pair(m, kk),
                        rhs=x_parts[kk][:, :, sl],
                        start=(kk == 0),
                        stop=(kk == 2),
                        perf_mode=DR,
                    )
                nc.vector.tensor_scalar(
                    rqk_sb[:, m, sl], qk_ps[:], 0.0, None, mybir.AluOpType.max
                )

            def fc_half(p, qs_sb, ks_sb, h):
                sl = slice(h * 512, (h + 1) * 512)
                for w_sb, r_m, out_sb, b_sb in (
                    (wfc1_sb, p, qs_sb, b1_sb),
                    (wfc2_sb, NPAIR + p, ks_sb, b2_sb),
                ):
                    f_ps = acc_tile()
                    nc.tensor.matmul(
                        f_ps[:], lhsT=w_sb[:], rhs=rqk_sb[:, r_m, sl],
                        start=True, stop=True,
                    )
                    nc.vector.tensor_scalar(
                        out_sb[:, sl], f_ps[:], b_sb[:, 0:1], None,
                        mybir.AluOpType.add,
                    )

            def mk_state(p):
                qs_sb = spk_pool.tile([128, N], BF16, tag="spk")
                ks_sb = spk_pool.tile([128, N], BF16, tag="spk")
                pt_t = pt_pool.tile([128, NT, 2, N], FP8, tag="pt")
                fc_half(p, qs_sb, ks_sb, 0)
                fc_half(p, qs_sb, ks_sb, 1)
                return (qs_sb, ks_sb, pt_t)

            def emit_v(nt, chunks=((0, 512), (512, 256))):
                # one token tile of the v projection -> vo fp8 head blocks
                for n0, nsz in chunks:
                    v_ps = acc_tile()
                    for kk in range(3):
                        nc.tensor.matmul(
                            v_ps[:, 0:nsz],
                            lhsT=x_parts[kk][:, :, nt * 128 : (nt + 1) * 128],
                            rhs=wv_sb[:, 2 * kk : 2 * kk + 2, n0 : n0 + nsz],
                            start=(kk == 0),
                            stop=(kk == 2),
                            perf_mode=DR,
                        )
                    h0 = n0 // HD
                    with nc.allow_low_precision(reason="relu(v) in fp8: errors average over 1024 attention positions, <0.1% on out"):
                        nc.vector.tensor_scalar(
                            vo_sb[:, nt, h0 : h0 + nsz // HD, 0:HD],
                            v_ps[:, 0:nsz],
                            0.0,
                            None,
                            mybir.AluOpType.max,
                        )

            def emit_colsum():
                # colsum(Vr) over all tokens: ones-lhsT chains -> [1, 960]
                # -> DRAM transpose bounce -> cva [65, 12] per-head columns
                cs_row = rs_pool.tile([128, 2, 480], F32, tag="csrow")
                for half in range(2):
                    cs_ps = acc_tile()
                    for nt in range(NT):
                        nc.tensor.matmul(
                            cs_ps[0:1, 0:480],
                            lhsT=ones8_sb[:],
                            rhs=vo_sb[:, nt, 6 * half : 6 * half + 6, :],
                            start=(nt == 0),
                            stop=(nt == NT - 1),
                        )
                    nc.vector.tensor_copy(
                        out=cs_row[0:1, half, :], in_=cs_ps[0:1, 0:480]
                    )
                nc.sync.dma_start(
                    cv_dram.rearrange("h c -> (h c)")[None, :], cs_row[0:1, :, :]
                )
                nc.sync.dma_start(
                    cva_sb[0:HD, :], cv_dram.rearrange("h c -> c h")[0:HD, :]
                )
                nc.vector.memset(cva_sb[HD : HD + 1, :], 0.0)

            def sc_mms(st, jt, h):
                qs_sb, ks_sb, _, s_A, s_B = st
                jsl = slice(jt * 128, (jt + 1) * 128)
                sl = slice(h * 512, (h + 1) * 512)
                for base, s_ps2 in ((0, s_A), (64, s_B)):
                    nc.tensor.matmul(
                        s_ps2[:, sl],
                        lhsT=ks_sb[base : base + 64, jsl],
                        rhs=qs_sb[base : base + 64, sl],
                        start=True, stop=True,
                    )

            def sc_finish(st, jt):
                _, _, pt_t, s_A, s_B = st
                t_sb = t_pool.tile([128, 2048], BF16, tag="texp")
                nc.scalar.activation(
                    t_sb[:, 0:1024], s_A[:], mybir.ActivationFunctionType.Exp
                )
                nc.scalar.activation(
                    t_sb[:, 1024:2048], s_B[:], mybir.ActivationFunctionType.Exp
                )
                with nc.allow_low_precision(reason="ptil=relu(exp(z)-1) in [0,0.09]: fp8e4m3 keeps ~6% relative step, <0.3% on out"):
                    nc.vector.tensor_scalar(
                        pt_t[:, jt], t_sb[:], -1.0, 0.0,
                        mybir.AluOpType.add, mybir.AluOpType.max,
                    )

            def emit_sc_fill(st, jt):
                s_A = psum.tile([128, N], F32, tag="ps")
                s_B = psum.tile([128, N], F32, tag="ps")
                st2 = (st[0], st[1], st[2], s_A, s_B)
                sc_mms(st2, jt, 0)
                sc_mms(st2, jt, 1)
                sc_finish(st2, jt)

            def emit_pv_h(p, h, heads=(0, 1)):
                # PV+rowsum for one i-half: 4 DoubleRow matmuls per head
                # (256 tokens/instr), V carries the ones column so row
                # 64 is rowsum(ptil). Copyback adds cva (colsum correction)
                # and stages to bf16; DMAs place rows 0:64 into outT (head
                # B shifted to partitions 64:128) and row 64 into rs_dram.
                pt_t = state[p][2]
                sl = slice(h * 512, (h + 1) * 512)
                for hh in heads:
                    head = 2 * p + hh
                    out_ps = acc_tile()
                    for jp in range(4):
                        nc.tensor.matmul(
                            out_ps[0:65, :],
                            lhsT=vo_sb[:, 2 * jp : 2 * jp + 2, head, 0:65],
                            rhs=pt_t[:, 2 * jp : 2 * jp + 2, hh, sl],
                            start=(jp == 0),
                            stop=(jp == 3),
                            perf_mode=DR,
                        )
                    stg = stg_pool.tile([128, 512], BF16, tag="stg")
                    nc.vector.tensor_scalar(
                        stg[0:65, :], out_ps[0:65, :],
                        cva_sb[0:65, head : head + 1], None,
                        mybir.AluOpType.add,
                    )
                    # staging hop on the idle gpsimd SWDGE queue; the
                    # latency-critical rs row stays on the sync queue
                    nc.gpsimd.dma_start(
                        outT_sb[64 * hh : 64 * hh + 64, p, sl], stg[0:64, :]
                    )
                    if p < NPAIR - 1:
                        nc.sync.dma_start(
                            rs_dram[p, hh, sl], stg[64:65, :]
                        )

            recbs = {}

            def emit_recip(p):
                rsq = rs_pool.tile([128, 16], BF16, tag="rsq")
                nc.sync.dma_start(
                    rsq[:], rs_dram[p].rearrange("h (pq t) -> h pq t", t=16)
                )
                rsf = rs_pool.tile([128, 16], F32, tag="rsf")
                nc.vector.tensor_scalar(
                    rsf[:], rsq[:], float(N), None, mybir.AluOpType.add
                )
                recq = rs_pool.tile([128, 16], BF16, tag="recq")
                with nc.allow_low_precision(reason="softmax denominators are O(1e3); bf16 recip adds <0.4% relative error"):
                    nc.vector.reciprocal(recq[:], rsf[:])
                nc.sync.dma_start(
                    rec_dram[p].rearrange("h (pq t) -> h pq t", t=16), recq[:]
                )
                recb = rs_pool.tile([128, N], BF16, tag="recb")
                for head in range(2):
                    for q in range(2):
                        nc.sync.dma_start(
                            recb[64 * head + 32 * q : 64 * head + 32 * q + 32, :],
                            rec_dram[p, head][None, :].to_broadcast([32, N]),
                        )
                recbs[p] = recb

            def emit_norm(p):
                recb = recbs[p]
                for h in range(NH):
                    sl = slice(h * 512, (h + 1) * 512)
                    nc.vector.tensor_tensor(
                        outT_sb[:, p, sl], outT_sb[:, p, sl], recb[:, sl],
                        mybir.AluOpType.mult,
                    )

            def emit_proj1(et):
                # proj partial sum over kc 0..2, bias folded in
                y_ps = ps_tile()
                for h in range(NH):
                    sl = slice(h * 512, (h + 1) * 512)
                    for kc in range(3):
                        nc.tensor.matmul(
                            y_ps[:, sl],
                            lhsT=wp_sb[:, kc, et * 128 : (et + 1) * 128],
                            rhs=outT_sb[:, kc, sl],
                            start=(kc == 0),
                            stop=(kc == 2),
                        )
                nc.vector.tensor_scalar(
                    y1_sb[:, et, :], y_ps[:], bproj_sb[:, et : et + 1], None,
                    mybir.AluOpType.add,
                )

            def emit_proj2a(et):
                y_ps = ps_tile()
                for h in range(NH):
                    sl = slice(h * 512, (h + 1) * 512)
                    for kc in (3, 4):
                        nc.tensor.matmul(
                            y_ps[:, sl],
                            lhsT=wp_sb[:, kc, et * 128 : (et + 1) * 128],
                            rhs=outT_sb[:, kc, sl],
                            start=(kc == 3),
                            stop=False,
                        )
                return y_ps

            def emit_proj2b(et, y_ps):
                for h in range(NH):
                    sl = slice(h * 512, (h + 1) * 512)
                    nc.tensor.matmul(
                        y_ps[:, sl],
                        lhsT=wp_sb[:, 5, et * 128 : (et + 1) * 128],
                        rhs=outT_sb[:, 5, sl],
                        start=False,
                        stop=True,
                    )
                y_sb = y_pool.tile([128, N], BF16, tag="yt")
                qs_out = (nc.sync, nc.scalar, nc.gpsimd)[et % 3]
                for h in range(NH):
                    sl = slice(h * 512, (h + 1) * 512)
                    with nc.allow_low_precision(reason="bf16 output: 0.4% rounding, well under the 2e-2 budget"):
                        nc.vector.tensor_tensor(
                            y_sb[:, sl], y_ps[:, sl], y1_sb[:, et, sl],
                            mybir.AluOpType.add,
                        )
                    qs_out.dma_start(yT_v[:, et, sl], y_sb[:, sl])

            # ---- prelude ----
            # PE warm-up on the first weight DMA while x half 0 lands:
            # flips the HAM clock gate to 8/8 before real work starts.
            def heartbeat(n):
                # dense trash matmuls: keep the HAM utilization monitor fed
                # so it never duty-cycles the PE clock to 4/8 mid-kernel
                for _ in range(n):
                    nc.tensor.matmul(
                        trash_holder[0][0:64, 0:128],
                        lhsT=wfc1_sb[0:128, 0:64],
                        rhs=wfc1_sb[:],
                        start=True,
                        stop=True,
                    )

            gate(wfc1_sb[:])
            heartbeat(64)
            gate(xA_sb[:, 0, 0:512])  # covers sync-queue consts + xA h0
            gate(xB_sb[:, 0, 0:512])  # covers scalar m0/m6 + xB h0
            gate(xC_sb[:, 0, 0:512])  # covers gpsimd xC h0

            # half-0 qk -> fc -> the h0 score matmuls of jt0/jt1, so the
            # first exp only waits on the x half-1 DMA + a short PE chain.
            for m in (0, NPAIR):
                emit_qk_half(m, 0)
            qs0 = spk_pool.tile([128, N], BF16, tag="spk")
            ks0 = spk_pool.tile([128, N], BF16, tag="spk")
            pt0 = pt_pool.tile([128, NT, 2, N], FP8, tag="pt")
            fc_half(0, qs0, ks0, 0)
            st0_scores = []
            for jt in range(2):
                s_A = psum.tile([128, N], F32, tag="ps")
                s_B = psum.tile([128, N], F32, tag="ps")
                st2 = (qs0, ks0, pt0, s_A, s_B)
                sc_mms(st2, jt, 0)
                st0_scores.append(st2)
            gate(xA_sb[:, 0, 512:N])
            gate(xB_sb[:, 0, 512:N])
            gate(xC_sb[:, 0, 512:N])
            for m in (0, NPAIR):
                emit_qk_half(m, 1)
            fc_half(0, qs0, ks0, 1)
            sc_mms(st0_scores[0], 0, 1)
            sc_finish(st0_scores[0], 0)
            sc_mms(st0_scores[1], 1, 1)
            sc_finish(st0_scores[1], 1)
            state = {0: (qs0, ks0, pt0)}
            # no gates for the gpsimd-queue weights: the scheduler hoists
            # gates ahead of the prelude and stalls the in-order PE queue
            # on the slow SWDGE DMAs; the framework's auto-inserted wait at
            # each first-consuming matmul lands exactly where needed.

            # ---- pipelined windows ----
            for w in range(NPAIR):
                slabs = []
                # front-load next pair's qk/fc so the next window's first
                # score fills never wait at the boundary
                if w + 1 < NPAIR:
                    for h in range(NH):
                        slabs.append((900, (
                            lambda m=w + 1, h=h: emit_qk_half(m, h))))
                        slabs.append((900, (
                            lambda m=NPAIR + w + 1, h=h: emit_qk_half(m, h))))
                    slabs.append((1000, (
                        lambda p=w + 1: state.__setitem__(p, mk_state(p)))))
                if w == 0:
                    for nt in range(0, 6):
                        slabs.append((1500, (
                            lambda nt=nt: emit_v(nt, ((0, 512),)))))
                        slabs.append((900, (
                            lambda nt=nt: emit_v(nt, ((512, 256),)))))
                if w == 1:
                    for nt in range(6, NT):
                        slabs.append((1500, (
                            lambda nt=nt: emit_v(nt, ((0, 512),)))))
                        slabs.append((900, (
                            lambda nt=nt: emit_v(nt, ((512, 256),)))))
                    slabs.append((1800, emit_colsum))

                # norms for pairs whose recb landed last window
                if w == 3:
                    slabs.append((400, (lambda: emit_norm(0))))
                    slabs.append((400, (lambda: emit_norm(1))))
                elif w >= 4:
                    slabs.append((400, (lambda p=w - 2: emit_norm(p))))

                # pv of prior pairs (w2 carries pairs 0 and 1)
                pv_list = [0, 1] if w == 2 else ([w - 1] if w >= 3 else [])
                for p in pv_list:
                    for h in range(NH):
                        for hh in range(2):
                            slabs.append((1150, (
                                lambda p=p, h=h, hh=hh:
                                emit_pv_h(p, h, (hh,)))))
                    slabs.append((300, (lambda p=p: emit_recip(p))))

                if w == 4:
                    for et in range(KC):
                        slabs.append((1500, (lambda et=et: emit_proj1(et))))
                if w == 5:
                    slabs.append((400, (lambda: emit_norm(4))))

                # pad the window's PE tail: the HAM down-clocks on idle
                nhb = 3 if w == 5 else 1
                for _ in range(nhb):
                    slabs.append((900, (lambda: heartbeat(8))))

                total = sum(c for c, _ in slabs)
                jts = range(2, NT) if w == 0 else range(NT)
                njt = len(jts)
                spent = 0
                done = 0.0
                for jt in jts:
                    emit_sc_fill(state[w], jt)
                    done += total / njt
                    while slabs and spent < done:
                        c, fn = slabs.pop(0)
                        fn()
                        spent += c
                for c, fn in slabs:
                    fn()
                if w >= 2:
                    for p in pv_list:
                        del state[p]

            # ---- tail ----
            # pair 5's reciprocal runs on the now-idle ACT engine instead
            # of the DMA reshape bounce: DR ones-matmul chains replicate
            # the rowsums into a [128,512] tile per half (A rows 0:64, B
            # 64:128), then rec = exp(-ln(x + N)) straight from PSUM.
            p5 = NPAIR - 1
            pt5 = state[p5][2]
            for h in range(NH):
                emit_pv_h(p5, h)
            rs5 = []
            for h in range(NH):
                sl = slice(h * 512, (h + 1) * 512)
                rs_ps = acc_tile()
                for hh in range(2):
                    for jp in range(4):
                        nc.tensor.matmul(
                            rs_ps[:, :],
                            lhsT=ones2_sb[:, :, 64 * hh : 64 * hh + 128],
                            rhs=pt5[:, 2 * jp : 2 * jp + 2, hh, sl],
                            start=(hh == 0 and jp == 0),
                            stop=(hh == 1 and jp == 3),
                            perf_mode=DR,
                        )
                rs5.append(rs_ps)
            y_stage = {et: emit_proj2a(et) for et in range(3)}
            heartbeat(8)
            recb5 = rs_pool.tile([128, N], BF16, tag="recb")
            with nc.allow_low_precision(reason="bf16 ln/exp of O(1e3) softmax denominators; <0.4% relative error"):
                lnt = rs_pool.tile([128, N], BF16, tag="lnt")
                for h in range(NH):
                    sl = slice(h * 512, (h + 1) * 512)
                    nc.scalar.activation(
                        lnt[:, sl], rs5[h][:],
                        mybir.ActivationFunctionType.Ln,
                        bias=n1024_sb[:, 0:1],
                    )
                nc.scalar.activation(
                    recb5[:], lnt[:],
                    mybir.ActivationFunctionType.Exp, scale=-1.0,
                )
            recbs[p5] = recb5
            emit_norm(p5)
            for et in range(KC):
                if et >= 3:
                    y_stage[et] = emit_proj2a(et)
                emit_proj2b(et, y_stage.pop(et))

    nc.compile()
    return nc


_NC_CACHE = {}


def _get_nc():
    if "nc" not in _NC_CACHE:
        _NC_CACHE["nc"] = build_nc()
    return _NC_CACHE["nc"]


def _make_in_maps(x, Wqkv, Wfc1, bfc1, Wfc2, bfc2, Wproj, bproj):
    bf = ml_dtypes.bfloat16
    f8 = ml_dtypes.float8_e4m3
    s2 = 2.0 * SCALE  # fold the *SCALE and the *N_HALF accumulation into Q path
    wqkv_scaled = Wqkv.copy()
    wqkv_scaled[2 * C :, :] *= float(T_STEPS)  # fold relu(v)*T into Wv
    wqkvT = np.ascontiguousarray(wqkv_scaled.T).astype(f8)
    wfc1p = np.zeros((128, 128), np.float32)
    wfc1p[0:64, 0:64] = Wfc1.T * s2
    wfc1p[64:128, 64:128] = Wfc1.T * s2
    wfc1p = wfc1p.astype(bf)
    wfc2p = np.zeros((128, 128), np.float32)
    wfc2p[0:64, 0:64] = Wfc2.T
    wfc2p[64:128, 64:128] = Wfc2.T
    wfc2p = wfc2p.astype(bf)
    b1p = np.concatenate([bfc1 * s2, bfc1 * s2]).astype(np.float32)[:, None]
    b2p = np.concatenate([bfc2, bfc2]).astype(np.float32)[:, None]
    wprojT = np.ascontiguousarray(Wproj.T).astype(bf)
    bprojp = np.ascontiguousarray(bproj.astype(np.float32).reshape(KC, 128).T)
    shared = dict(
        wqkvT=wqkvT, wfc1p=np.ascontiguousarray(wfc1p),
        wfc2p=np.ascontiguousarray(wfc2p), b1p=b1p, b2p=b2p,
        wprojT=wprojT, bprojp=bprojp,
    )
    maps = []
    for b in range(B):
        m = dict(shared)
        m["xT"] = np.ascontiguousarray(x[b].T).astype(f8)
        maps.append(m)
    return maps


def kernel(**inputs) -> np.ndarray:
    x = np.asarray(inputs["x"], dtype=np.float32)
    nc = _get_nc()
    in_maps = _make_in_maps(
        x,
        np.asarray(inputs["Wqkv"], np.float32),
        np.asarray(inputs["Wfc1"], np.float32),
        np.asarray(inputs["bfc1"], np.float32),
        np.asarray(inputs["Wfc2"], np.float32),
        np.asarray(inputs["bfc2"], np.float32),
        np.asarray(inputs["Wproj"], np.float32),
        np.asarray(inputs["bproj"], np.float32),
    )
    res = run_bass_kernel_spmd(nc, in_maps, core_ids=list(range(B)))
    return assemble_out(res)


def assemble_out(res) -> np.ndarray:
    out = np.empty((B, N, C), dtype=np.float32)
    for b in range(B):
        out[b] = np.asarray(res.results[b]["yT"], dtype=np.float32).T
    return out
